# revision 10
# baseline (speedup 1.0000x reference)
"""Gaussian RBF kernel for Trainium2, data-parallel over batch across 8 cores.

exp(-0.5*||x-mu||^2/sigma^2) folded into ONE augmented GEMM + Exp:
  E[s,o] = sum_d x[s,d]*(2*a[o]*mus[o,d]) + x2[s]*(-a[o]) + 1*(-a[o]*m2[o])
with a = 0.5/sigma^2.  Augmented contraction K = D+2 = 66; the tiny weight
matrix W (66,512) and the x2/ones augmentation are built on host.

Per core: xaT (66,4096) @ W -> (4096,512) via 32 matmuls of [66,128]x[66,512]
into PSUM using float32r (full-rate PE: 1 cycle/row vs 4 for plain fp32),
Exp on the scalar engine in [128,2048] chunks (4 PSUM banks per ACTIVATE
amortizes the ~352-cycle pipeline fill).  Output must stay fp32: the entire
result tensor is in the f32-denormal range (max ~1e-43), so 16-bit formats
flush it to zero.  Output DMAs move 2 MiB each ([128,8192] fp32 -> a
(p,t,o)-rearranged [S,O] DRAM view, 2 KiB contiguous runs).

Raw bass engine programs (explicit semaphores) — the Tile framework's
attached-wait sync scheme trips "Too many sync wait commands" in this
compiler build, so engines are programmed directly.

The builder takes a repeat count R (default 1): the whole pipeline,
including input DMAs (double-buffered xaT), is replayed R times so bench
harnesses can measure steady-state HW time via the R-slope.
"""
import numpy as np
from concourse import bass, mybir
from concourse import bass_utils

B, S, D, O = 8, 4096, 64, 512
K = D + 2          # 66: [x, x2, 1]
P = 128            # rows (s) per matmul tile
NT = S // P        # 32 tiles
TPC = 4            # tiles per ACT chunk (= 4 PSUM banks = 2048 fp32)
NCH = NT // TPC    # 8 ACT chunks per iteration
CH = 4             # sbuf output ring: 4 chunks of [128, 2048] fp32
CPD = 2            # ACT chunks per output DMA (2 MiB fp32)
NDMA = NCH // CPD  # 4 output DMAs per iteration

FP = mybir.dt.float32
FR = mybir.dt.float32r

# The true args are all <= -99, so exp() lands entirely in the f32-denormal
# range, which the ACT engine flushes to zero.  Shift the exponent by +SHIFT
# inside the GEMM (folded into W's constant row) so the device computes
# exp(arg+SHIFT) in normal f32 range; the host multiplies by exp(-SHIFT) in
# float64, which rounds to the correct f32 denormals.  y-values that flush
# to zero under the shift correspond to outputs < 2e-66 — zero in the f32
# reference as well.
SHIFT = 64.0


def _build(R=1):
    nc = bass.Bass()
    xaT = nc.declare_dram_parameter("xaT", [K, S], FR, isOutput=False)
    w = nc.declare_dram_parameter("w", [K, O], FR, isOutput=False)
    out = nc.declare_dram_parameter("out", [S, O], FP, isOutput=True)

    H = S // 2  # xaT DMA'd in halves for earlier PE start

    with (
        nc.sbuf_tensor([K, 2 * S], FR) as xt,     # double-buffered input
        nc.sbuf_tensor([K, O], FR) as wt,
        nc.sbuf_tensor([P, CH * TPC * O], FP) as ot,
        nc.psum_tensor([P, 2 * TPC * O], FP) as ps,  # all 8 banks
        nc.Block() as block,
        nc.semaphore("dma_in") as dma_in,
        nc.semaphore("mm") as mm,
        nc.semaphore("act_s") as act_s,
        nc.semaphore("dma_out") as dma_out,
    ):
        @block.sync
        def _(sync):
            sync.dma_start(out=wt[:], in_=w[:]).then_inc(dma_in, 16)
            for it in range(R):
                if it == 0:
                    for half in range(2):
                        sync.dma_start(
                            out=xt[:, half * H:(half + 1) * H],
                            in_=xaT[:, half * H:(half + 1) * H],
                        ).then_inc(dma_in, 16)
                if it + 1 < R:
                    # prefetch next iteration's input into the other buffer;
                    # safe once iteration it-1 finished reading it
                    if it >= 1:
                        sync.wait_ge(mm, NT * it)
                    b = (it + 1) % 2
                    for half in range(2):
                        sync.dma_start(
                            out=xt[:, b * S + half * H:b * S + (half + 1) * H],
                            in_=xaT[:, half * H:(half + 1) * H],
                        ).then_inc(dma_in, 16)
                for dd in range(NDMA):
                    d = it * NDMA + dd          # global DMA index
                    g = 2 * d                   # first global chunk it covers
                    sync.wait_ge(act_s, g + CPD)
                    sb = (g % CH) * TPC * O
                    rows = CPD * TPC * P        # s-rows per DMA (1024)
                    dram = out[dd * rows:(dd + 1) * rows, :].rearrange(
                        "(t p) o -> p t o", p=P
                    )
                    sync.dma_start(
                        out=dram,
                        in_=ot[:, sb:sb + CPD * TPC * O],
                    ).then_inc(dma_out, 16)
            sync.wait_ge(dma_out, 16 * NDMA * R)

        @block.tensor
        def _(pe):
            for it in range(R):
                b = it % 2
                for t in range(NT):
                    if t == 0:
                        pe.wait_ge(dma_in, 32 * it + 32)
                    elif t == NT // 2:
                        pe.wait_ge(dma_in, 32 * it + 48)
                    g = it * NCH + t // TPC     # global chunk this tile feeds
                    if g >= 2:
                        # psum bank group reuse: chunk g-2 must be drained
                        pe.wait_ge(act_s, g - 1)
                    pe.matmul(
                        ps[:, (t % (2 * TPC)) * O:(t % (2 * TPC) + 1) * O],
                        xt[:, b * S + t * P:b * S + (t + 1) * P],
                        wt[:],
                        start=True,
                        stop=True,
                    ).then_inc(mm, 1)

        @block.scalar
        def _(scalar):
            for it in range(R):
                for c in range(NCH):
                    g = it * NCH + c
                    scalar.wait_ge(mm, it * NT + (c + 1) * TPC)
                    if g >= CH:
                        # sbuf ring slot reuse: occupant chunk g-CH must be
                        # DMA'd out (it went in DMA (g-CH)//CPD)
                        scalar.wait_ge(dma_out, 16 * ((g - CH) // CPD + 1))
                    sb = (g % CH) * TPC * O
                    scalar.activation(
                        ot[:, sb:sb + TPC * O],
                        ps[:, (c % 2) * TPC * O:(c % 2 + 1) * TPC * O],
                        mybir.ActivationFunctionType.Exp,
                    ).then_inc(act_s, 1)

    return nc


def _host_inputs(x, mus, log_sigmas):
    a = 0.5 * np.exp(-2.0 * log_sigmas.astype(np.float64))          # (O,)
    m2 = np.sum(mus.astype(np.float64) ** 2, axis=1)                # (O,)
    W = np.empty((K, O), np.float32)
    W[:D] = (2.0 * a[None, :] * mus.T.astype(np.float64)).astype(np.float32)
    W[D] = (-a).astype(np.float32)
    W[D + 1] = (-a * m2 + SHIFT).astype(np.float32)

    x2 = np.sum(x.astype(np.float64) * x.astype(np.float64), axis=-1)
    in_maps = []
    for i in range(B):
        xa = np.empty((S, K), np.float32)
        xa[:, :D] = x[i]
        xa[:, D] = x2[i]
        xa[:, D + 1] = 1.0
        in_maps.append({"xaT": np.ascontiguousarray(xa.T), "w": W})
    return in_maps


def kernel(x, mus, log_sigmas):
    x = np.asarray(x, np.float32)
    mus = np.asarray(mus, np.float32)
    log_sigmas = np.asarray(log_sigmas, np.float32)

    in_maps = _host_inputs(x, mus, log_sigmas)
    nc = _build()
    res = bass_utils.run_bass_kernel_spmd(nc, in_maps, list(range(B)))
    global LAST_RESULT
    LAST_RESULT = res
    scale = np.exp(np.float64(-SHIFT))
    return np.stack(
        [
            (np.asarray(r["out"]).astype(np.float64) * scale).astype(np.float32)
            for r in res.results
        ],
        axis=0,
    )


LAST_RESULT = None


# revision 14
# speedup vs baseline: 1.1452x; 1.1452x over previous
"""Gaussian RBF kernel for Trainium2, data-parallel over batch across 8 cores.

exp(-0.5*||x-mu||^2/sigma^2) folded into ONE augmented GEMM + Exp:
  E[s,o] = sum_d x[s,d]*(2*a[o]*mus[o,d]) + x2[s]*(-a[o]) + 1*(-a[o]*m2[o]+SHIFT)
with a = 0.5/sigma^2.  Augmented contraction K = D+2 = 66; the tiny weight
matrix W (66,512) and the x2/ones augmentation are built on host.

Per core: xaT (66,4096) @ W -> (4096,512) via 32 matmuls of [66,128]x[66,512]
into PSUM using float32r (full-rate PE: 1 cycle/row vs 4 for plain fp32),
Exp on the scalar engine in [128,2048] chunks (4 PSUM banks per ACTIVATE
amortizes the ~352-cycle pipeline fill).

Output must stay fp32: the true exp() results all fall in the f32-denormal
range (args <= -99), which both 16-bit formats and the ACT engine flush to
zero.  SHIFT=+64 is folded into the GEMM so the device computes
exp(arg+64) in normal f32 range; the host multiplies by exp(-64) in float64
which rounds to the correct f32 denormals.

DMA strategy (TRN2 has two HWDGE rings, one for the sync (SP) queue and one
for the ACT queue; DMAs on one ring execute FIFO with ~2-4us issue gaps):
  - input DMAs (W + xaT in halves) issue from the ACT queue at t=0, before
    its activations start;
  - each ACT chunk's 1 MiB output DMA alternates rings: even chunks from
    sync, odd chunks issued inline on the ACT queue right after the chunk's
    activation (no semaphore needed);
  - single-shot latency helpers: ~16 tiny dummy matmuls warm the PE HAM
    clock gate during the input load, and a 1-element dummy Exp forces the
    ACT table load off the critical path.

Raw bass engine programs (explicit semaphores) — the Tile framework's
attached-wait sync scheme trips "Too many sync wait commands" in this
compiler build, so engines are programmed directly.

The builder takes a repeat count R (default 1): the whole pipeline,
including input DMAs (double-buffered xaT), is replayed R times so bench
harnesses can measure steady-state HW time via the R-slope.
"""
import numpy as np
from concourse import bass, mybir
from concourse import bass_utils

B, S, D, O = 8, 4096, 64, 512
K = D + 2          # 66: [x, x2, 1]
P = 128            # rows (s) per matmul tile
NT = S // P        # 32 tiles
TPC = 4            # tiles per ACT chunk (= 4 PSUM banks = 2048 fp32)
NCH = NT // TPC    # 8 ACT chunks per iteration
CH = 4             # sbuf output ring: 4 chunks of [128, 2048] fp32
NWARM = 16         # PE HAM warmup matmuls

FP = mybir.dt.float32
FR = mybir.dt.float32r

# exponent shift: see module docstring
SHIFT = 64.0


def _build(R=1):
    nc = bass.Bass()
    xaT = nc.declare_dram_parameter("xaT", [K, S], FR, isOutput=False)
    w = nc.declare_dram_parameter("w", [K, O], FR, isOutput=False)
    out = nc.declare_dram_parameter("out", [S, O], FP, isOutput=True)

    H = S // 2  # xaT DMA'd in halves for earlier PE start

    with (
        nc.sbuf_tensor([K, 2 * S], FR) as xt,     # double-buffered input
        nc.sbuf_tensor([K, O], FR) as wt,
        nc.sbuf_tensor([P, CH * TPC * O], FP) as ot,
        nc.sbuf_tensor([P, P], FR) as scr,        # warmup scratch (never DMA'd)
        nc.sbuf_tensor([P, 4], FP) as scrf,       # ACT table-preload scratch
        nc.psum_tensor([P, 2 * TPC * O], FP) as ps,  # all 8 banks
        nc.Block() as block,
        nc.semaphore("dma_in") as dma_in,
        nc.semaphore("mm") as mm,
        nc.semaphore("act_s") as act_s,
        nc.semaphore("dma_out_a") as dma_out_a,   # sync-ring output DMAs
        nc.semaphore("dma_out_b") as dma_out_b,   # ACT-ring output DMAs
    ):
        OSEM = (dma_out_a, dma_out_b)

        def out_dma(q, g):
            # 1 MiB: chunk g's [128, 2048] f32 slot -> (p,t,o) view of out
            sb = (g % CH) * TPC * O
            c = g % NCH
            rows = TPC * P
            dram = out[c * rows:(c + 1) * rows, :].rearrange(
                "(t p) o -> p t o", p=P
            )
            q.dma_start(out=dram, in_=ot[:, sb:sb + TPC * O]).then_inc(
                OSEM[g % 2], 16
            )

        @block.sync
        def _(sync):
            for it in range(R):
                for c in range(0, NCH, 2):      # even chunks on the sync ring
                    g = it * NCH + c
                    sync.wait_ge(act_s, g + 1)
                    out_dma(sync, g)
            sync.wait_ge(dma_out_a, 16 * (NCH // 2) * R)
            sync.wait_ge(dma_out_b, 16 * (NCH // 2) * R)

        @block.tensor
        def _(pe):
            for _ in range(NWARM):              # HAM warmup on scratch data
                pe.matmul(
                    ps[:, :64], scr[:K, :], scr[:K, :64],
                    start=True, stop=True,
                )

        # (shapes: lhsT [66,128] -> out partitions 128; rhs [66,64] -> free 64)
            for it in range(R):
                b = it % 2
                for t in range(NT):
                    if t == 0:
                        pe.wait_ge(dma_in, 32 * it + 32)
                    elif t == NT // 2:
                        pe.wait_ge(dma_in, 32 * it + 48)
                    g = it * NCH + t // TPC     # global chunk this tile feeds
                    if g >= 2:
                        # psum bank group reuse: chunk g-2 must be drained
                        pe.wait_ge(act_s, g - 1)
                    pe.matmul(
                        ps[:, (t % (2 * TPC)) * O:(t % (2 * TPC) + 1) * O],
                        xt[:, b * S + t * P:b * S + (t + 1) * P],
                        wt[:],
                        start=True,
                        stop=True,
                    ).then_inc(mm, 1)

        @block.scalar
        def _(scalar):
            # input DMAs ride this queue's HWDGE ring, ahead of the ACTs
            scalar.dma_start(out=wt[:], in_=w[:]).then_inc(dma_in, 16)
            for half in range(2):
                scalar.dma_start(
                    out=xt[:, half * H:(half + 1) * H],
                    in_=xaT[:, half * H:(half + 1) * H],
                ).then_inc(dma_in, 16)
            # dummy Exp: walrus inserts the ACT table load before it, so the
            # ~2.7us load overlaps the input DMAs
            scalar.activation(
                scrf[:, 2:3], scrf[:, 0:1], mybir.ActivationFunctionType.Exp
            )
            for it in range(R):
                if it + 1 < R:
                    # prefetch next iteration's input into the other buffer;
                    # safe once iteration it-1 finished reading it
                    if it >= 1:
                        scalar.wait_ge(mm, NT * it)
                    b = (it + 1) % 2
                    for half in range(2):
                        scalar.dma_start(
                            out=xt[:, b * S + half * H:b * S + (half + 1) * H],
                            in_=xaT[:, half * H:(half + 1) * H],
                        ).then_inc(dma_in, 16)
                for c in range(NCH):
                    g = it * NCH + c
                    scalar.wait_ge(mm, it * NT + (c + 1) * TPC)
                    if g >= CH:
                        # sbuf ring slot reuse: occupant chunk g-CH must have
                        # been DMA'd out (ring parity (g-CH)%2)
                        d = g - CH
                        scalar.wait_ge(OSEM[d % 2], 16 * (d // 2 + 1))
                    sb = (g % CH) * TPC * O
                    scalar.activation(
                        ot[:, sb:sb + TPC * O],
                        ps[:, (c % 2) * TPC * O:(c % 2 + 1) * TPC * O],
                        mybir.ActivationFunctionType.Exp,
                    ).then_inc(act_s, 1)
                    if c % 2 == 1:
                        # odd chunks: DMA from this queue's ring.  The wait on
                        # our own act_s increment is required: the engine
                        # pipelines instructions, and only the semaphore
                        # increment signals that the activation's SBUF writes
                        # have landed.
                        scalar.wait_ge(act_s, g + 1)
                        out_dma(scalar, g)

    return nc


def _host_inputs(x, mus, log_sigmas):
    a = 0.5 * np.exp(-2.0 * log_sigmas.astype(np.float64))          # (O,)
    m2 = np.sum(mus.astype(np.float64) ** 2, axis=1)                # (O,)
    W = np.empty((K, O), np.float32)
    W[:D] = (2.0 * a[None, :] * mus.T.astype(np.float64)).astype(np.float32)
    W[D] = (-a).astype(np.float32)
    W[D + 1] = (-a * m2 + SHIFT).astype(np.float32)

    x2 = np.sum(x.astype(np.float64) * x.astype(np.float64), axis=-1)
    in_maps = []
    for i in range(B):
        xa = np.empty((S, K), np.float32)
        xa[:, :D] = x[i]
        xa[:, D] = x2[i]
        xa[:, D + 1] = 1.0
        in_maps.append({"xaT": np.ascontiguousarray(xa.T), "w": W})
    return in_maps


def kernel(x, mus, log_sigmas):
    x = np.asarray(x, np.float32)
    mus = np.asarray(mus, np.float32)
    log_sigmas = np.asarray(log_sigmas, np.float32)

    in_maps = _host_inputs(x, mus, log_sigmas)
    nc = _build()
    res = bass_utils.run_bass_kernel_spmd(nc, in_maps, list(range(B)))
    global LAST_RESULT
    LAST_RESULT = res
    scale = np.exp(np.float64(-SHIFT))
    return np.stack(
        [
            (np.asarray(r["out"]).astype(np.float64) * scale).astype(np.float32)
            for r in res.results
        ],
        axis=0,
    )


LAST_RESULT = None
